# revision 23
# baseline (speedup 1.0000x reference)
"""CustomLSTM cell on 8 TRN2 NeuronCores — fp8/fp16 hybrid matmul.

Data-parallel over batch (4096 rows/core). Precision plan (error budget
2e-2; this config measures eh=1.963e-2 / ec=1.462e-2 on HW, matching
the CPU simulation to 4 digits):
  - i, f gates: full-K fp8e4m3 DoubleRow matmuls (2x PE rate)
  - o gate:     fp8 DR on K 0:768 (all cols) + K 768:1024 cols 0:448;
                fp16 on K 768:1024 cols 448:512
  - g (tanh) gate: full fp16 (1x rate, 8x less quant error than bf16)
  - epilogue entirely in fp16 (gates, cell, c, h): fp16 rounding is
    error-negligible (2.8e-4 rel vs the 2e-2 budget), DVE runs 16-bit
    ops at 2x, and the h/c output DMA bytes halve vs f32/bf16 pairs.
All W are premultiplied by 16 on the host (fp8 subnormal avoidance);
activations fold the 1/16 dequant into their scale operand.

PE floor: 20.25 bf16-units = 138.3us/core (vs 143.5 for the old mix).
Matmuls fused to full 512-col instructions: 22 matmul+LDW pairs per
128-row btile instead of 32. Startup input DMAs are chunked in
consumption order; the
epilogue is reordered (o's activation before g's, tanh(c) on the h
path only) so the tail after the final matmul is short.

Self-contained: shapes/sharding hardcoded for
input [32768, 1024], cell_state [32768, 512], W* [1024, 512].
"""

import os

import numpy as np
import ml_dtypes

import bass_rust
import concourse.bass as bass
import concourse.mybir as mybir
import concourse.tile as tile
from concourse.bass_utils import run_bass_kernel_spmd

N_CORES = 8
B = 32768
D = 1024
H = 512
P = 128
B_LOC = B // N_CORES        # 4096 rows per core
KO = D // P                 # 8 k-subtiles of 128
KS = KO // 2                # 4 DoubleRow k-steps of 256
NW = 4 * H                  # 2048 concatenated gate dim
NG = 4
BG_ROWS = 512               # batch rows per x slab
BG = B_LOC // BG_ROWS       # 8 slabs per core
BT_PER_BG = BG_ROWS // P    # 4 batch tiles per slab

OC8 = 448                   # o-gate fp8 column coverage on k6..7
WSCALE = 16.0               # host premultiplies all W; undone in activation
N_WARM = 16                 # PE p-state warmup matmuls

F8 = mybir.dt.float8e4
F16 = mybir.dt.float16
BF16 = mybir.dt.bfloat16
F32 = mybir.dt.float32
NPF8 = ml_dtypes.float8_e4m3
NPF16 = np.float16
NPBF = ml_dtypes.bfloat16

LAST_RESULTS = None
_CACHED = {}


def _split_multi_waits(nc):
    """Legalize for a walrus build that accepts one sync-wait per instruction."""
    n = 0
    for f in nc.m.functions:
        for blk in f.blocks:
            insts = blk.instructions
            if not any(
                i.sync_info is not None and len(i.sync_info.on_wait) > 1
                for i in insts
            ):
                continue
            out = []
            for inst in insts:
                si = inst.sync_info
                if si is not None and len(si.on_wait) > 1:
                    waits = list(si.on_wait)
                    for w in waits[:-1]:
                        nop = mybir.InstNoOp(name=f"waitsplit_{n}", ins=[], outs=[])
                        n += 1
                        nop.engine = inst.engine
                        nop.sync_info = bass_rust.SyncInfo(on_wait=[w], on_update=[])
                        out.append(nop)
                    inst.sync_info = bass_rust.SyncInfo(
                        on_wait=[waits[-1]], on_update=list(si.on_update)
                    )
                out.append(inst)
            blk.instructions = out


class _FastTailTileContext(tile.TileContext):
    """Drop both tail all-engine barriers.

    The stock tail is [drain+waits][barrier][sem/queue reset][barrier].  The
    drain's sem waits already cover completion of every instruction and DMA,
    so by the time the gpsimd-side reset runs nothing is in flight that could
    observe the cleared semaphores; NRT waits for each engine stream to halt
    independently.  Saves ~8-10us of EVSEM barrier ring.
    """

    def _drain_and_barrier(self, tick_clock, wait_clock):
        drain_inst = self.nc.sync.drain()
        tail_sem = self.nc.alloc_semaphore("fast_tail_sem")
        drain_inst.then_inc(tail_sem)
        self.nc.gpsimd.wait_ge(tail_sem, 1)
        assert self.sems is not None
        popped = self.nc._tile_sem_poison_stack.pop()
        assert popped is self._sem_poison
        self.nc.clear_and_free_semaphores(list(self.sems.allocated().values()))


def _build(with_bias):
    nc = bass.Bass()
    AF = mybir.ActivationFunctionType
    ts = bass.ts
    DR = mybir.MatmulPerfMode.DoubleRow
    SCL = 1.0 / WSCALE

    x8t = nc.dram_tensor("x8t", [BG, P, KO, BG_ROWS], F8, kind="ExternalInput")
    xht = nc.dram_tensor("xht", [BG, P, KO, BG_ROWS], F16, kind="ExternalInput")
    w8if = nc.dram_tensor("w8if", [P, KO, 2 * H], F8, kind="ExternalInput")  # i|f
    w8o = nc.dram_tensor("w8o", [P, KO, H], F8, kind="ExternalInput")
    wg = nc.dram_tensor("wg", [P, KO, H], F16, kind="ExternalInput")
    wo16 = nc.dram_tensor("wo16", [P, 2, H - OC8], F16, kind="ExternalInput")
    cell = nc.dram_tensor("cell", [B_LOC, H], F16, kind="ExternalInput")
    if with_bias:
        bias = nc.dram_tensor("bias", [P, NW], F32, kind="ExternalInput")
    hc_out = nc.dram_tensor("hc_out", [B_LOC, 2, H], F16, kind="ExternalOutput")

    with _FastTailTileContext(nc) as tc:
        with (
            tc.tile_pool(name="wpool", bufs=1) as wpool,
            tc.tile_pool(name="x8pool", bufs=3) as x8pool,
            tc.tile_pool(name="xhpool", bufs=3) as xhpool,
            tc.tile_pool(name="cpool", bufs=4) as cpool,
            tc.tile_pool(name="gpool", bufs=3) as gpool,
            tc.tile_pool(name="ppool", bufs=8, space="PSUM") as ppool,
        ):
            # PE warmup: memset on the vector engine (fast to start, unlike
            # gpsimd) so the HAM clock gate opens while the startup DMA is
            # still in flight.  Short enough that the PE is free again by the
            # time the first real chunk lands.
            wz = wpool.tile([P, P], F8, tag="wz", name="wz")
            nc.vector.memset(wz[:], 0.0)
            warm_ps = ppool.tile([P, P], F32, tag="ps", name="warm_ps")
            for _ in range(N_WARM):
                nc.tensor.matmul(warm_ps[:], wz[:], wz[:], start=True, stop=True)

            bias_t = None
            if with_bias:
                bias_t = wpool.tile([P, NW], F32, tag="bias_t", name="bias_t")
                nc.sync.dma_start(bias_t[:], bias[:])

            # Resident weight tiles + slab-0 x, DMA'd in k-pair chunks in
            # exact consumption order: the first matmul only needs the first
            # (x8 k0-1, w8if k0-1) chunk, ~384KB, instead of a half-slab.
            w8t = wpool.tile([P, KO, 2 * H], F8, tag="w8t", name="w8t")
            w8ot = wpool.tile([P, KO, H], F8, tag="w8ot", name="w8ot")
            wgt = wpool.tile([P, KO, H], F16, tag="wgt", name="wgt")
            wo16t = wpool.tile([P, 2, H - OC8], F16, tag="wo16t", name="wo16t")
            x8s0 = wpool.tile([P, KO, BG_ROWS], F8, tag="x8s0", name="x8s0")
            xhs0 = wpool.tile([P, KO, BG_ROWS], F16, tag="xhs0", name="xhs0")
            cts0 = [
                cpool.tile([P, H], F16, tag="ct", name=f"ct0_{j}")
                for j in range(BT_PER_BG)
            ]
            nc.sync.dma_start(x8s0[:, 0:2, :], x8t[0, :, 0:2, :])
            nc.sync.dma_start(w8t[:, 0:2, :H], w8if[:, 0:2, :H])
            nc.sync.dma_start(w8t[:, 0:2, H:], w8if[:, 0:2, H:])
            nc.sync.dma_start(w8ot[:, 0:2, :], w8o[:, 0:2, :])
            for ks in range(1, KS):
                kk = slice(2 * ks, 2 * ks + 2)
                nc.sync.dma_start(x8s0[:, kk, :], x8t[0, :, kk, :])
                nc.sync.dma_start(w8t[:, kk, :], w8if[:, kk, :])
                nc.sync.dma_start(w8ot[:, kk, :], w8o[:, kk, :])
            for kp in range(KS):
                kk = slice(2 * kp, 2 * kp + 2)
                nc.sync.dma_start(xhs0[:, kk, :], xht[0, :, kk, :])
                nc.sync.dma_start(wgt[:, kk, :], wg[:, kk, :])
            nc.sync.dma_start(wo16t[:], wo16[:])
            for j in range(BT_PER_BG):
                nc.sync.dma_start(cts0[j][:], cell[j * P : (j + 1) * P, :])

            def mm_fp8(ps, x8s, j, ks):
                """One DoubleRow k-step (256 K) for i, f, o off a shared
                stationary x tile.  ks==3 covers only o's first OC8 cols."""
                l8 = x8s[:, 2 * ks : 2 * ks + 2, ts(j, P)]
                for gi in (0, 1):
                    nc.tensor.matmul(
                        ps[gi],
                        l8,
                        w8t[:, 2 * ks : 2 * ks + 2, gi * H : (gi + 1) * H],
                        start=(ks == 0),
                        stop=(ks == KS - 1),
                        perf_mode=DR,
                        skip_group_check=True,
                    )
                if ks < KS - 1:
                    nc.tensor.matmul(
                        ps[3],
                        l8,
                        w8ot[:, 2 * ks : 2 * ks + 2, :],
                        start=(ks == 0),
                        stop=False,
                        perf_mode=DR,
                        skip_group_check=True,
                    )
                else:
                    nc.tensor.matmul(
                        ps[3][:, :OC8],
                        l8,
                        w8ot[:, 2 * ks : 2 * ks + 2, :OC8],
                        start=False,
                        stop=False,
                        perf_mode=DR,
                        skip_group_check=True,
                    )

            def mm_o16(ps, xhs, j, k):
                nc.tensor.matmul(
                    ps[3][:, OC8:],
                    xhs[:, k, ts(j, P)],
                    wo16t[:, k - 6, :],
                    start=False,
                    stop=(k == KO - 1),
                    skip_group_check=True,
                )

            def mm_g(ps, xhs, j, k):
                nc.tensor.matmul(
                    ps[2], xhs[:, k, ts(j, P)], wgt[:, k, :],
                    start=(k == 0), stop=(k == KO - 1),
                )

            def mm_btile(ps, x8s, xhs, j):
                # steady slabs: o's fp16 sliver right after the fp8 phase, so
                # the o bank completes before g's long fp16 run — the
                # epilogue's o activation then only waits on g.
                for ks in range(KS):
                    mm_fp8(ps, x8s, j, ks)
                for k in (6, 7):
                    mm_o16(ps, xhs, j, k)
                for k in range(KO):
                    mm_g(ps, xhs, j, k)

            def epilogue(ps, ct, rows, uid, splits=1):
                # psum order: 0=i 1=f 2=g 3=o; scale undoes the 16x W premul.
                # Scalar order i,f,o,g,tn: everything not needing g runs
                # before g's bank closes; h's chain after the last matmul is
                # just act_g -> cn -> tn -> hn.
                if with_bias:
                    zs = []
                    for nn in range(NG):
                        z = gpool.tile([P, H], F32, tag=f"z{nn}", name=f"z{nn}_{uid}")
                        nc.vector.tensor_add(z[:], ps[nn], bias_t[:, ts(nn, H)])
                        zs.append(z)
                else:
                    zs = ps
                w_ = H // splits
                for q in range(splits):
                    cs = slice(q * w_, (q + 1) * w_)
                    i_t = gpool.tile([P, w_], F16, tag="i_t", name=f"i_{uid}_{q}")
                    nc.scalar.activation(i_t[:], zs[0][:, cs], AF.Sigmoid, scale=SCL)
                    f_t = gpool.tile([P, w_], F16, tag="f_t", name=f"f_{uid}_{q}")
                    nc.scalar.activation(f_t[:], zs[1][:, cs], AF.Sigmoid, scale=SCL)
                    o_t = gpool.tile([P, w_], F16, tag="o_t", name=f"o_{uid}_{q}")
                    nc.scalar.activation(o_t[:], zs[3][:, cs], AF.Sigmoid, scale=SCL)
                    g_t = gpool.tile([P, w_], F16, tag="g_t", name=f"g_{uid}_{q}")
                    nc.scalar.activation(g_t[:], zs[2][:, cs], AF.Tanh, scale=SCL)

                    fc = gpool.tile([P, w_], F16, tag="fc", name=f"fc_{uid}_{q}")
                    nc.vector.tensor_mul(fc[:], f_t[:], ct[:, cs])
                    ig = gpool.tile([P, w_], F16, tag="ig", name=f"ig_{uid}_{q}")
                    nc.vector.tensor_mul(ig[:], i_t[:], g_t[:])
                    chn = gpool.tile([P, 2, w_], F16, tag="chn", name=f"chn_{uid}_{q}")
                    nc.vector.tensor_add(chn[:, 0, :], fc[:], ig[:])
                    tn = gpool.tile([P, w_], F16, tag="tn", name=f"tn_{uid}_{q}")
                    nc.scalar.activation(tn[:], chn[:, 0, :], AF.Tanh)
                    nc.vector.tensor_mul(chn[:, 1, :], o_t[:], tn[:])
                    nc.sync.dma_start(hc_out[rows, :, cs], chn[:])

            def epilogue_last(ps, ct, rows, uid):
                """Final btile only: same instructions as epilogue(splits=2)
                but with all early activations (i/f/o of both column halves)
                front-loaded so they pre-run during this btile's own matmuls,
                instead of queuing behind the first half's tanh chain.
                Purely intra-btile emission reordering."""
                w_ = H // 2
                pre = []
                for q in (0, 1):
                    cs = slice(q * w_, (q + 1) * w_)
                    i_t = gpool.tile([P, w_], F16, tag="i_t", name=f"i_{uid}_{q}")
                    nc.scalar.activation(i_t[:], ps[0][:, cs], AF.Sigmoid, scale=SCL)
                    f_t = gpool.tile([P, w_], F16, tag="f_t", name=f"f_{uid}_{q}")
                    nc.scalar.activation(f_t[:], ps[1][:, cs], AF.Sigmoid, scale=SCL)
                    o_t = gpool.tile([P, w_], F16, tag="o_t", name=f"o_{uid}_{q}")
                    nc.scalar.activation(o_t[:], ps[3][:, cs], AF.Sigmoid, scale=SCL)
                    pre.append((cs, i_t, f_t, o_t))
                gs = []
                for q in (0, 1):
                    cs = pre[q][0]
                    g_t = gpool.tile([P, w_], F16, tag="g_t", name=f"g_{uid}_{q}")
                    nc.scalar.activation(g_t[:], ps[2][:, cs], AF.Tanh, scale=SCL)
                    gs.append(g_t)
                chns = []
                for q in (0, 1):
                    cs, i_t, f_t, o_t = pre[q]
                    fc = gpool.tile([P, w_], F16, tag="fc", name=f"fc_{uid}_{q}")
                    nc.vector.tensor_mul(fc[:], f_t[:], ct[:, cs])
                    ig = gpool.tile([P, w_], F16, tag="ig", name=f"ig_{uid}_{q}")
                    nc.vector.tensor_mul(ig[:], i_t[:], gs[q][:])
                    chn = gpool.tile([P, 2, w_], F16, tag="chn", name=f"chn_{uid}_{q}")
                    nc.vector.tensor_add(chn[:, 0, :], fc[:], ig[:])
                    chns.append((chn, o_t, cs))
                tns = []
                for q in (0, 1):
                    chn, o_t, cs = chns[q]
                    tn = gpool.tile([P, w_], F16, tag="tn", name=f"tn_{uid}_{q}")
                    nc.scalar.activation(tn[:], chn[:, 0, :], AF.Tanh)
                    tns.append(tn)
                for q in (0, 1):
                    chn, o_t, cs = chns[q]
                    nc.vector.tensor_mul(chn[:, 1, :], o_t[:], tns[q][:])
                    nc.sync.dma_start(hc_out[rows, :, cs], chn[:])

            # ---- slab 0: j-pairs, phases interleaved with the chunked DMA
            # stream's arrival order (fp8 k-pairs, then fp16 k-pairs, o16
            # sliver last since wo16/xh k6-7 arrive last).
            xslabs = {}

            def prefetch_slab(g):
                x8s = x8pool.tile([P, KO, BG_ROWS], F8, tag="x8s", name=f"x8s_{g}")
                nc.sync.dma_start(x8s[:], x8t[g])
                xhs = xhpool.tile([P, KO, BG_ROWS], F16, tag="xhs", name=f"xhs_{g}")
                nc.sync.dma_start(xhs[:], xht[g])
                xslabs[g] = (x8s, xhs)

            for jp in (0, 2):
                ps2 = {
                    (j, nn): ppool.tile([P, H], F32, tag="ps", name=f"ps0_{j}_{nn}")
                    for j in (jp, jp + 1)
                    for nn in range(NG)
                }
                for ks in range(KS):
                    for j in (jp, jp + 1):
                        mm_fp8([ps2[(j, nn)] for nn in range(NG)], x8s0, j, ks)
                for k in range(KO):
                    for j in (jp, jp + 1):
                        mm_g([ps2[(j, nn)] for nn in range(NG)], xhs0, j, k)
                    if k >= 6:
                        for j in (jp, jp + 1):
                            mm_o16([ps2[(j, nn)] for nn in range(NG)], xhs0, j, k)
                for j in (jp, jp + 1):
                    epilogue(
                        [ps2[(j, nn)] for nn in range(NG)],
                        cts0[j],
                        slice(j * P, (j + 1) * P),
                        f"g0_{j}",
                    )
                    if j == 1:
                        prefetch_slab(1)
                    elif j == 3:
                        prefetch_slab(2)

            # ---- slabs 1..7 against prefetched slabs.  Cell tiles load one
            # btile ahead so their triggers clear the out-DMA triggers.
            ct_next = cpool.tile([P, H], F16, tag="ct", name="ct_4")
            nc.sync.dma_start(ct_next[:], cell[4 * P : 5 * P, :])
            for g in range(1, BG):
                x8s, xhs = xslabs[g]
                for j in range(BT_PER_BG):
                    bt = g * BT_PER_BG + j
                    rows = slice(bt * P, (bt + 1) * P)
                    ct = ct_next
                    ps = [
                        ppool.tile([P, H], F32, tag="ps", name=f"ps{nn}_{bt}")
                        for nn in range(NG)
                    ]
                    mm_btile(ps, x8s, xhs, j)
                    if bt + 1 < BG * BT_PER_BG:
                        ct_next = cpool.tile(
                            [P, H], F16, tag="ct", name=f"ct_{bt + 1}"
                        )
                        nc.sync.dma_start(
                            ct_next[:], cell[(bt + 1) * P : (bt + 2) * P, :]
                        )
                    last_bt = g == BG - 1 and j == BT_PER_BG - 1
                    if last_bt and not with_bias:
                        epilogue_last(ps, ct, rows, f"g{g}_{j}")
                    else:
                        epilogue(ps, ct, rows, f"g{g}_{j}")
                    if j == 0 and g + 1 < BG:
                        prefetch_slab(g + 1)

    _split_multi_waits(nc)
    return nc


def kernel(input, cell_state, Wi, bi, Wf, bf, Wg, bg, Wo, bo):
    global LAST_RESULTS

    x = np.asarray(input, dtype=np.float32)
    cell = np.ascontiguousarray(np.asarray(cell_state, dtype=np.float32).astype(NPF16))
    Wi, Wf, Wg, Wo = (np.asarray(m, dtype=np.float32) for m in (Wi, Wf, Wg, Wo))
    bcat = np.concatenate(
        [np.asarray(v, dtype=np.float32) for v in (bi, bf, bg, bo)]
    )  # [4H] in i,f,g,o order (matches psum order)
    with_bias = bool(np.any(bcat))

    def wlay(Wcat, np_dt):
        # [D, N] -> [p, ko, n], scaled by WSCALE
        n = Wcat.shape[1]
        return np.ascontiguousarray(
            (Wcat * WSCALE).astype(np_dt).reshape(KO, P, n).transpose(1, 0, 2)
        )

    w8if_dev = wlay(np.concatenate([Wi, Wf], axis=1), NPF8)       # [P,KO,1024]
    w8o_dev = wlay(Wo, NPF8)                                      # [P,KO,512]
    wg_dev = wlay(Wg, NPF16)                                      # [P,KO,512]
    wo16_dev = np.ascontiguousarray(
        (Wo[6 * P :, OC8:] * WSCALE).astype(NPF16).reshape(2, P, H - OC8)
        .transpose(1, 0, 2)
    )  # [P,2,64]

    in_maps = []
    for c in range(N_CORES):
        xc = x[c * B_LOC : (c + 1) * B_LOC]  # [4096, 1024]

        def xlay(np_dt):
            return np.ascontiguousarray(
                xc.astype(np_dt)
                .reshape(BG, BG_ROWS, KO, P)
                .transpose(0, 3, 2, 1)
            )

        m = {
            "x8t": xlay(NPF8),
            "xht": xlay(NPF16),
            "w8if": w8if_dev,
            "w8o": w8o_dev,
            "wg": wg_dev,
            "wo16": wo16_dev,
            "cell": cell[c * B_LOC : (c + 1) * B_LOC],
        }
        if with_bias:
            m["bias"] = np.ascontiguousarray(
                np.broadcast_to(bcat[None, :] * WSCALE, (P, NW)).astype(np.float32)
            )
        in_maps.append(m)

    key = with_bias
    if key not in _CACHED:
        _CACHED[key] = _build(with_bias)
    nc = _CACHED[key]

    trace = os.environ.get("KERNEL_TRACE", "0") == "1"
    res = run_bass_kernel_spmd(nc, in_maps, list(range(N_CORES)), trace=trace)
    LAST_RESULTS = res

    hc = np.concatenate(
        [res.results[c]["hc_out"] for c in range(N_CORES)], axis=0
    ).astype(np.float32)
    return hc[:, 1, :], hc[:, 0, :]


# revision 24
# speedup vs baseline: 1.0003x; 1.0003x over previous
"""CustomLSTM cell on 8 TRN2 NeuronCores — fp8/fp16 hybrid matmul.

Data-parallel over batch (4096 rows/core). Precision plan (error budget
2e-2; this config measures eh=1.963e-2 / ec=1.462e-2 on HW, matching
the CPU simulation to 4 digits):
  - i, f gates: full-K fp8e4m3 DoubleRow matmuls (2x PE rate)
  - o gate:     fp8 DR on K 0:768 (all cols) + K 768:1024 cols 0:448;
                fp16 on K 768:1024 cols 448:512
  - g (tanh) gate: full fp16 (1x rate, 8x less quant error than bf16)
  - epilogue entirely in fp16 (gates, cell, c, h): fp16 rounding is
    error-negligible (2.8e-4 rel vs the 2e-2 budget), DVE runs 16-bit
    ops at 2x, and the h/c output DMA bytes halve vs f32/bf16 pairs.
All W are premultiplied by 16 on the host (fp8 subnormal avoidance);
activations fold the 1/16 dequant into their scale operand.

PE floor: 20.25 bf16-units = 138.3us/core (vs 143.5 for the old mix).
Matmuls fused to full 512-col instructions: 22 matmul+LDW pairs per
128-row btile instead of 32. Startup input DMAs are chunked in
consumption order; the
epilogue is reordered (o's activation before g's, tanh(c) on the h
path only) so the tail after the final matmul is short.

Self-contained: shapes/sharding hardcoded for
input [32768, 1024], cell_state [32768, 512], W* [1024, 512].
"""

import os

import numpy as np
import ml_dtypes

import bass_rust
import concourse.bass as bass
import concourse.mybir as mybir
import concourse.tile as tile
from concourse.bass_utils import run_bass_kernel_spmd

N_CORES = 8
B = 32768
D = 1024
H = 512
P = 128
B_LOC = B // N_CORES        # 4096 rows per core
KO = D // P                 # 8 k-subtiles of 128
KS = KO // 2                # 4 DoubleRow k-steps of 256
NW = 4 * H                  # 2048 concatenated gate dim
NG = 4
BG_ROWS = 512               # batch rows per x slab
BG = B_LOC // BG_ROWS       # 8 slabs per core
BT_PER_BG = BG_ROWS // P    # 4 batch tiles per slab

OC8 = 448                   # o-gate fp8 column coverage on k6..7
WSCALE = 16.0               # host premultiplies all W; undone in activation
N_WARM = 16                 # PE p-state warmup matmuls

F8 = mybir.dt.float8e4
F16 = mybir.dt.float16
BF16 = mybir.dt.bfloat16
F32 = mybir.dt.float32
NPF8 = ml_dtypes.float8_e4m3
NPF16 = np.float16
NPBF = ml_dtypes.bfloat16

LAST_RESULTS = None
_CACHED = {}


def _split_multi_waits(nc):
    """Legalize for a walrus build that accepts one sync-wait per instruction."""
    n = 0
    for f in nc.m.functions:
        for blk in f.blocks:
            insts = blk.instructions
            if not any(
                i.sync_info is not None and len(i.sync_info.on_wait) > 1
                for i in insts
            ):
                continue
            out = []
            for inst in insts:
                si = inst.sync_info
                if si is not None and len(si.on_wait) > 1:
                    waits = list(si.on_wait)
                    for w in waits[:-1]:
                        nop = mybir.InstNoOp(name=f"waitsplit_{n}", ins=[], outs=[])
                        n += 1
                        nop.engine = inst.engine
                        nop.sync_info = bass_rust.SyncInfo(on_wait=[w], on_update=[])
                        out.append(nop)
                    inst.sync_info = bass_rust.SyncInfo(
                        on_wait=[waits[-1]], on_update=list(si.on_update)
                    )
                out.append(inst)
            blk.instructions = out


class _FastTailTileContext(tile.TileContext):
    """Drop both tail all-engine barriers.

    The stock tail is [drain+waits][barrier][sem/queue reset][barrier].  The
    drain's sem waits already cover completion of every instruction and DMA,
    so by the time the gpsimd-side reset runs nothing is in flight that could
    observe the cleared semaphores; NRT waits for each engine stream to halt
    independently.  Saves ~8-10us of EVSEM barrier ring.
    """

    def _drain_and_barrier(self, tick_clock, wait_clock):
        drain_inst = self.nc.sync.drain()
        tail_sem = self.nc.alloc_semaphore("fast_tail_sem")
        drain_inst.then_inc(tail_sem)
        self.nc.gpsimd.wait_ge(tail_sem, 1)
        assert self.sems is not None
        popped = self.nc._tile_sem_poison_stack.pop()
        assert popped is self._sem_poison
        self.nc.clear_and_free_semaphores(list(self.sems.allocated().values()))


def _build(with_bias):
    nc = bass.Bass()
    AF = mybir.ActivationFunctionType
    ts = bass.ts
    DR = mybir.MatmulPerfMode.DoubleRow
    SCL = 1.0 / WSCALE

    x8t = nc.dram_tensor("x8t", [BG, P, KO, BG_ROWS], F8, kind="ExternalInput")
    xht = nc.dram_tensor("xht", [BG, P, KO, BG_ROWS], F16, kind="ExternalInput")
    w8if = nc.dram_tensor("w8if", [P, KO, 2 * H], F8, kind="ExternalInput")  # i|f
    w8o = nc.dram_tensor("w8o", [P, KO, H], F8, kind="ExternalInput")
    wg = nc.dram_tensor("wg", [P, KO, H], F16, kind="ExternalInput")
    wo16 = nc.dram_tensor("wo16", [P, 2, H - OC8], F16, kind="ExternalInput")
    cell = nc.dram_tensor("cell", [B_LOC, H], F16, kind="ExternalInput")
    if with_bias:
        bias = nc.dram_tensor("bias", [P, NW], F32, kind="ExternalInput")
    hc_out = nc.dram_tensor("hc_out", [B_LOC, 2, H], F16, kind="ExternalOutput")

    with _FastTailTileContext(nc) as tc:
        with (
            tc.tile_pool(name="wpool", bufs=1) as wpool,
            tc.tile_pool(name="x8pool", bufs=3) as x8pool,
            tc.tile_pool(name="xhpool", bufs=3) as xhpool,
            tc.tile_pool(name="cpool", bufs=4) as cpool,
            tc.tile_pool(name="gpool", bufs=3) as gpool,
            tc.tile_pool(name="ppool", bufs=8, space="PSUM") as ppool,
        ):
            # PE warmup: memset on the vector engine (fast to start, unlike
            # gpsimd) so the HAM clock gate opens while the startup DMA is
            # still in flight.  Short enough that the PE is free again by the
            # time the first real chunk lands.
            wz = wpool.tile([P, P], F8, tag="wz", name="wz")
            nc.vector.memset(wz[:], 0.0)
            warm_ps = ppool.tile([P, P], F32, tag="ps", name="warm_ps")
            for _ in range(N_WARM):
                nc.tensor.matmul(warm_ps[:], wz[:], wz[:], start=True, stop=True)

            bias_t = None
            if with_bias:
                bias_t = wpool.tile([P, NW], F32, tag="bias_t", name="bias_t")
                nc.sync.dma_start(bias_t[:], bias[:])

            # Resident weight tiles + slab-0 x, DMA'd in k-pair chunks in
            # exact consumption order: the first matmul only needs the first
            # (x8 k0-1, w8if k0-1) chunk, ~384KB, instead of a half-slab.
            w8t = wpool.tile([P, KO, 2 * H], F8, tag="w8t", name="w8t")
            w8ot = wpool.tile([P, KO, H], F8, tag="w8ot", name="w8ot")
            wgt = wpool.tile([P, KO, H], F16, tag="wgt", name="wgt")
            wo16t = wpool.tile([P, 2, H - OC8], F16, tag="wo16t", name="wo16t")
            x8s0 = wpool.tile([P, KO, BG_ROWS], F8, tag="x8s0", name="x8s0")
            xhs0 = wpool.tile([P, KO, BG_ROWS], F16, tag="xhs0", name="xhs0")
            cts0 = [
                cpool.tile([P, H], F16, tag="ct", name=f"ct0_{j}")
                for j in range(BT_PER_BG)
            ]
            nc.sync.dma_start(x8s0[:, 0:2, :], x8t[0, :, 0:2, :])
            nc.sync.dma_start(w8t[:, 0:2, :H], w8if[:, 0:2, :H])
            nc.sync.dma_start(w8t[:, 0:2, H:], w8if[:, 0:2, H:])
            nc.sync.dma_start(w8ot[:, 0:2, :], w8o[:, 0:2, :])
            for ks in range(1, KS):
                kk = slice(2 * ks, 2 * ks + 2)
                nc.sync.dma_start(x8s0[:, kk, :], x8t[0, :, kk, :])
                nc.sync.dma_start(w8t[:, kk, :], w8if[:, kk, :])
                nc.sync.dma_start(w8ot[:, kk, :], w8o[:, kk, :])
            for kp in range(KS):
                kk = slice(2 * kp, 2 * kp + 2)
                nc.sync.dma_start(xhs0[:, kk, :], xht[0, :, kk, :])
                nc.sync.dma_start(wgt[:, kk, :], wg[:, kk, :])
            nc.sync.dma_start(wo16t[:], wo16[:])
            for j in range(BT_PER_BG):
                nc.sync.dma_start(cts0[j][:], cell[j * P : (j + 1) * P, :])

            def mm_fp8(ps, x8s, j, ks):
                """One DoubleRow k-step (256 K) for i, f, o off a shared
                stationary x tile.  ks==3 covers only o's first OC8 cols."""
                l8 = x8s[:, 2 * ks : 2 * ks + 2, ts(j, P)]
                for gi in (0, 1):
                    nc.tensor.matmul(
                        ps[gi],
                        l8,
                        w8t[:, 2 * ks : 2 * ks + 2, gi * H : (gi + 1) * H],
                        start=(ks == 0),
                        stop=(ks == KS - 1),
                        perf_mode=DR,
                        skip_group_check=True,
                    )
                if ks < KS - 1:
                    nc.tensor.matmul(
                        ps[3],
                        l8,
                        w8ot[:, 2 * ks : 2 * ks + 2, :],
                        start=(ks == 0),
                        stop=False,
                        perf_mode=DR,
                        skip_group_check=True,
                    )
                else:
                    nc.tensor.matmul(
                        ps[3][:, :OC8],
                        l8,
                        w8ot[:, 2 * ks : 2 * ks + 2, :OC8],
                        start=False,
                        stop=False,
                        perf_mode=DR,
                        skip_group_check=True,
                    )

            def mm_o16(ps, xhs, j, k):
                nc.tensor.matmul(
                    ps[3][:, OC8:],
                    xhs[:, k, ts(j, P)],
                    wo16t[:, k - 6, :],
                    start=False,
                    stop=(k == KO - 1),
                    skip_group_check=True,
                )

            def mm_g(ps, xhs, j, k):
                nc.tensor.matmul(
                    ps[2], xhs[:, k, ts(j, P)], wgt[:, k, :],
                    start=(k == 0), stop=(k == KO - 1),
                )

            def mm_btile(ps, x8s, xhs, j):
                # steady slabs: o's fp16 sliver right after the fp8 phase, so
                # the o bank completes before g's long fp16 run — the
                # epilogue's o activation then only waits on g.
                for ks in range(KS):
                    mm_fp8(ps, x8s, j, ks)
                for k in (6, 7):
                    mm_o16(ps, xhs, j, k)
                for k in range(KO):
                    mm_g(ps, xhs, j, k)

            def epilogue(ps, ct, rows, uid, splits=1):
                # psum order: 0=i 1=f 2=g 3=o; scale undoes the 16x W premul.
                # Scalar order i,f,o,g,tn: everything not needing g runs
                # before g's bank closes; h's chain after the last matmul is
                # just act_g -> cn -> tn -> hn.
                if with_bias:
                    zs = []
                    for nn in range(NG):
                        z = gpool.tile([P, H], F32, tag=f"z{nn}", name=f"z{nn}_{uid}")
                        nc.vector.tensor_add(z[:], ps[nn], bias_t[:, ts(nn, H)])
                        zs.append(z)
                else:
                    zs = ps
                w_ = H // splits
                for q in range(splits):
                    cs = slice(q * w_, (q + 1) * w_)
                    i_t = gpool.tile([P, w_], F16, tag="i_t", name=f"i_{uid}_{q}")
                    nc.scalar.activation(i_t[:], zs[0][:, cs], AF.Sigmoid, scale=SCL)
                    f_t = gpool.tile([P, w_], F16, tag="f_t", name=f"f_{uid}_{q}")
                    nc.scalar.activation(f_t[:], zs[1][:, cs], AF.Sigmoid, scale=SCL)
                    o_t = gpool.tile([P, w_], F16, tag="o_t", name=f"o_{uid}_{q}")
                    nc.scalar.activation(o_t[:], zs[3][:, cs], AF.Sigmoid, scale=SCL)
                    g_t = gpool.tile([P, w_], F16, tag="g_t", name=f"g_{uid}_{q}")
                    nc.scalar.activation(g_t[:], zs[2][:, cs], AF.Tanh, scale=SCL)

                    fc = gpool.tile([P, w_], F16, tag="fc", name=f"fc_{uid}_{q}")
                    nc.vector.tensor_mul(fc[:], f_t[:], ct[:, cs])
                    ig = gpool.tile([P, w_], F16, tag="ig", name=f"ig_{uid}_{q}")
                    nc.vector.tensor_mul(ig[:], i_t[:], g_t[:])
                    chn = gpool.tile([P, 2, w_], F16, tag="chn", name=f"chn_{uid}_{q}")
                    nc.vector.tensor_add(chn[:, 0, :], fc[:], ig[:])
                    tn = gpool.tile([P, w_], F16, tag="tn", name=f"tn_{uid}_{q}")
                    nc.scalar.activation(tn[:], chn[:, 0, :], AF.Tanh)
                    nc.vector.tensor_mul(chn[:, 1, :], o_t[:], tn[:])
                    nc.sync.dma_start(hc_out[rows, :, cs], chn[:])

            # ---- slab 0: j-pairs, phases interleaved with the chunked DMA
            # stream's arrival order (fp8 k-pairs, then fp16 k-pairs, o16
            # sliver last since wo16/xh k6-7 arrive last).
            xslabs = {}

            def prefetch_slab(g):
                x8s = x8pool.tile([P, KO, BG_ROWS], F8, tag="x8s", name=f"x8s_{g}")
                nc.sync.dma_start(x8s[:], x8t[g])
                xhs = xhpool.tile([P, KO, BG_ROWS], F16, tag="xhs", name=f"xhs_{g}")
                nc.sync.dma_start(xhs[:], xht[g])
                xslabs[g] = (x8s, xhs)

            for jp in (0, 2):
                ps2 = {
                    (j, nn): ppool.tile([P, H], F32, tag="ps", name=f"ps0_{j}_{nn}")
                    for j in (jp, jp + 1)
                    for nn in range(NG)
                }
                for ks in range(KS):
                    for j in (jp, jp + 1):
                        mm_fp8([ps2[(j, nn)] for nn in range(NG)], x8s0, j, ks)
                for k in range(KO):
                    for j in (jp, jp + 1):
                        mm_g([ps2[(j, nn)] for nn in range(NG)], xhs0, j, k)
                    if k >= 6:
                        for j in (jp, jp + 1):
                            mm_o16([ps2[(j, nn)] for nn in range(NG)], xhs0, j, k)
                for j in (jp, jp + 1):
                    epilogue(
                        [ps2[(j, nn)] for nn in range(NG)],
                        cts0[j],
                        slice(j * P, (j + 1) * P),
                        f"g0_{j}",
                    )
                    if j == 1:
                        prefetch_slab(1)
                    elif j == 3:
                        prefetch_slab(2)

            # ---- slabs 1..7 against prefetched slabs.  Cell tiles load one
            # btile ahead so their triggers clear the out-DMA triggers.
            ct_next = cpool.tile([P, H], F16, tag="ct", name="ct_4")
            nc.sync.dma_start(ct_next[:], cell[4 * P : 5 * P, :])
            for g in range(1, BG):
                x8s, xhs = xslabs[g]
                for j in range(BT_PER_BG):
                    bt = g * BT_PER_BG + j
                    rows = slice(bt * P, (bt + 1) * P)
                    ct = ct_next
                    ps = [
                        ppool.tile([P, H], F32, tag="ps", name=f"ps{nn}_{bt}")
                        for nn in range(NG)
                    ]
                    mm_btile(ps, x8s, xhs, j)
                    if bt + 1 < BG * BT_PER_BG:
                        ct_next = cpool.tile(
                            [P, H], F16, tag="ct", name=f"ct_{bt + 1}"
                        )
                        nc.sync.dma_start(
                            ct_next[:], cell[(bt + 1) * P : (bt + 2) * P, :]
                        )
                    last_bt = g == BG - 1 and j == BT_PER_BG - 1
                    epilogue(ps, ct, rows, f"g{g}_{j}", splits=2 if last_bt else 1)
                    if j == 0 and g + 1 < BG:
                        prefetch_slab(g + 1)

    _split_multi_waits(nc)
    return nc


def kernel(input, cell_state, Wi, bi, Wf, bf, Wg, bg, Wo, bo):
    global LAST_RESULTS

    x = np.asarray(input, dtype=np.float32)
    cell = np.ascontiguousarray(np.asarray(cell_state, dtype=np.float32).astype(NPF16))
    Wi, Wf, Wg, Wo = (np.asarray(m, dtype=np.float32) for m in (Wi, Wf, Wg, Wo))
    bcat = np.concatenate(
        [np.asarray(v, dtype=np.float32) for v in (bi, bf, bg, bo)]
    )  # [4H] in i,f,g,o order (matches psum order)
    with_bias = bool(np.any(bcat))

    def wlay(Wcat, np_dt):
        # [D, N] -> [p, ko, n], scaled by WSCALE
        n = Wcat.shape[1]
        return np.ascontiguousarray(
            (Wcat * WSCALE).astype(np_dt).reshape(KO, P, n).transpose(1, 0, 2)
        )

    w8if_dev = wlay(np.concatenate([Wi, Wf], axis=1), NPF8)       # [P,KO,1024]
    w8o_dev = wlay(Wo, NPF8)                                      # [P,KO,512]
    wg_dev = wlay(Wg, NPF16)                                      # [P,KO,512]
    wo16_dev = np.ascontiguousarray(
        (Wo[6 * P :, OC8:] * WSCALE).astype(NPF16).reshape(2, P, H - OC8)
        .transpose(1, 0, 2)
    )  # [P,2,64]

    in_maps = []
    for c in range(N_CORES):
        xc = x[c * B_LOC : (c + 1) * B_LOC]  # [4096, 1024]

        def xlay(np_dt):
            return np.ascontiguousarray(
                xc.astype(np_dt)
                .reshape(BG, BG_ROWS, KO, P)
                .transpose(0, 3, 2, 1)
            )

        m = {
            "x8t": xlay(NPF8),
            "xht": xlay(NPF16),
            "w8if": w8if_dev,
            "w8o": w8o_dev,
            "wg": wg_dev,
            "wo16": wo16_dev,
            "cell": cell[c * B_LOC : (c + 1) * B_LOC],
        }
        if with_bias:
            m["bias"] = np.ascontiguousarray(
                np.broadcast_to(bcat[None, :] * WSCALE, (P, NW)).astype(np.float32)
            )
        in_maps.append(m)

    key = with_bias
    if key not in _CACHED:
        _CACHED[key] = _build(with_bias)
    nc = _CACHED[key]

    trace = os.environ.get("KERNEL_TRACE", "0") == "1"
    res = run_bass_kernel_spmd(nc, in_maps, list(range(N_CORES)), trace=trace)
    LAST_RESULTS = res

    hc = np.concatenate(
        [res.results[c]["hc_out"] for c in range(N_CORES)], axis=0
    ).astype(np.float32)
    return hc[:, 1, :], hc[:, 0, :]
